# revision 6
# baseline (speedup 1.0000x reference)
"""Trainium2 Bass kernel for the Kalman graphical-model message-passing problem.

reference math (B=64, D=8, M=4, S=50000):
    m1 = -Qinv @ (xs - F @ x_past)            (B, D, S)
    m2 = FtQinv @ (x_fut - F @ xs)            (B, D, S)
    m3 = HtRinv @ ys_t - (HtRinv @ H) @ xs    (B, D, S)
with x_past/x_fut edge-replicated 1-sample shifts of xs along S.

Design v2 ("skewed single-pass", everything bf16 on the wire; rel-err gate
is 2e-2, bf16 end-to-end measures ~7.5e-3):

  * All three outputs come from ONE matmul per column chunk against ONE
    static lhsT.  Per group the rhs partition stack is
        v[c] = [x[T-1] (8); x[T] (8); y[T] (4)]        T = g*gw + c - 1
    and the output stack is [m1[T]; m2[T-1]; m3[T]]:
        m1[T]   = B1 x[T-1] + A1 x[T]
        m2[T-1] = A2 x[T-1] + B2 x[T]     <- the skew: m2 pops out shifted
        m3[T]   = A3 x[T]   + C3 y[T]        one column; its store AP just
                                             reads one column to the right.
    With 24 output states per group, NG=5 groups fill 120/128 PSUM
    partitions; contraction is 100 partitions.  PE cost: gw+1 = 10001
    columns per batch (vs 18750 for the 6-matmul variant) and a single
    LDWEIGHTS for the whole kernel (vs 6 swaps per chunk).

  * v-tile partition layout is [past 0-39 | cur 40-79 | y 80-99]: cur and
    y are host-packed and DMA'd; past is built on-device by one shifted
    copy per batch (KERNEL_SHIFT=dve: DVE tensor_copy writing at partition
    base 0; KERNEL_SHIFT=dma: SBUF->SBUF DMA) -- no extra HBM bytes.

  * Per batch: 2 load DMAs (x 800KB, y 400KB) + 6 store DMAs (2 column
    segments x 3 outputs, 10-20KB contiguous runs).  Loads on the ACT
    HWDGE ring, m1 stores on SP, m2 on the gpsimd SWDGE ring, m3
    alternates SP/SWDGE per batch to balance bytes (~9.6MB per ring).

  * PSUM: [120, 2048] fp32 tiles (4 banks) x 2 bufs = all 8 banks; four
    512-col matmuls fill a tile, one copy drains it (cast to bf16),
    alternating DVE/ACT to split the drain load.
"""

import os
from contextlib import ExitStack

import ml_dtypes
import numpy as np

import concourse.bacc as bacc
import concourse.bass as bass
import concourse.mybir as mybir
import concourse.tile as tile
from concourse.bass_utils import run_bass_kernel_spmd

F32 = mybir.dt.float32
BF16 = mybir.dt.bfloat16
NPBF16 = ml_dtypes.bfloat16

B, D, M, S = 64, 8, 4, 50000
N_CORES = 8
BC = B // N_CORES  # batches per core
NG = 5             # groups: 5 x 24 output states = 120 PSUM partitions
MW = 512           # matmul free-dim (one PSUM bank of fp32)
PW = 4 * MW        # psum tile width (4 banks)


def _geom(s):
    assert s % NG == 0, s
    gw = s // NG       # samples per group
    xc = gw + 2        # image cols c in [0, gw+2); col c ~ sample g*gw+c-1
    return gw, xc


def _build_nc(bc=BC, s=S):
    variant = os.environ.get("KERNEL_VARIANT", "full")  # perf bisection only
    shift = os.environ.get("KERNEL_SHIFT", "dma")       # dma | dve (dve is
    # rejected by the BIR verifier for 40-row slots: engine partition bases
    # must be 32-aligned; the SBUF->SBUF DMA shift has no such constraint)
    gw, xc = _geom(s)

    nc = bacc.Bacc(trn_type="TRN2")
    xp = nc.dram_tensor("xp", [bc, 40, xc], BF16, kind="ExternalInput")
    yp = nc.dram_tensor("yp", [bc, 20, xc], BF16, kind="ExternalInput")
    w = nc.dram_tensor("w_all", [100, 128], BF16, kind="ExternalInput")
    m_all = nc.dram_tensor("m_all", [bc, D, 3, s], BF16, kind="ExternalOutput")

    with tile.TileContext(nc) as tc, ExitStack() as ctx:
        singles = ctx.enter_context(tc.tile_pool(name="singles", bufs=1))
        vpool = ctx.enter_context(tc.tile_pool(name="v", bufs=3))
        opool = ctx.enter_context(tc.tile_pool(name="o", bufs=3))
        ppool = ctx.enter_context(tc.tile_pool(name="pp", bufs=2, space="PSUM"))

        w_sb = singles.tile([100, 128], BF16, tag="w")
        nc.sync.dma_start(out=w_sb[:], in_=w[:, :])

        for b in range(bc):
            v_t = vpool.tile([100, xc], BF16, tag="v", name=f"v_{b}")
            nc.scalar.dma_start(
                out=v_t[40:80, :], in_=bass.AP(xp, b * 40 * xc, [[xc, 40], [1, xc]])
            )
            nc.scalar.dma_start(
                out=v_t[80:100, :], in_=bass.AP(yp, b * 20 * xc, [[xc, 20], [1, xc]])
            )
            if variant == "loads":
                continue
            # past slots (rows 0-39): cur shifted one column right (cur col 0
            # already holds the clipped sample, so past[c] = cur[c-1]).
            if shift == "dve":
                nc.vector.tensor_copy(
                    out=v_t[0:40, 1:xc], in_=v_t[40:80, 0 : xc - 1]
                )
            else:
                eng = nc.sync if b % 2 == 0 else nc.gpsimd
                eng.dma_start(out=v_t[0:40, 1:xc], in_=v_t[40:80, 0 : xc - 1])

            o_t = opool.tile([120, xc], BF16, tag="o", name=f"o_{b}")

            n_cols = gw + 1  # compute cols c in [1, gw+2)
            pt = 0
            for c0 in range(1, 1 + n_cols, PW):
                tw = min(PW, 1 + n_cols - c0)
                p = ppool.tile([120, PW], F32, tag="p", name=f"p_{b}_{c0}")
                for h0 in range(0, tw, MW):
                    hw = min(MW, tw - h0)
                    nc.tensor.matmul(
                        p[:, h0 : h0 + hw],
                        w_sb[0:100, 0:120],
                        v_t[:, c0 + h0 : c0 + h0 + hw],
                        start=True,
                        stop=True,
                    )
                if pt % 2 == 0:
                    nc.vector.tensor_copy(out=o_t[:, c0 : c0 + tw], in_=p[:, 0:tw])
                else:
                    nc.scalar.copy(out=o_t[:, c0 : c0 + tw], in_=p[:, 0:tw])
                pt += 1

            if variant == "nostores":
                continue
            ooff = b * D * 3 * s
            # two column segments per output so segment A streams while the
            # tail chunks still compute; m3's ring alternates per batch.
            seg = min(max(PW, (gw // (2 * PW)) * PW), gw)  # segment-A width
            for o_idx, rows, csh in ((0, (0, 40), 1), (1, (40, 80), 2), (2, (80, 120), 1)):
                if o_idx == 0:
                    eng = nc.sync
                elif o_idx == 1:
                    eng = nc.gpsimd
                else:
                    eng = nc.sync if b % 2 else nc.gpsimd
                for t0, width in ((0, seg), (seg, gw - seg)):
                    if width == 0:
                        continue
                    eng.dma_start(
                        out=bass.AP(
                            m_all,
                            ooff + o_idx * s + t0,
                            [[gw, NG], [3 * s, D], [1, width]],
                        ),
                        in_=o_t[rows[0] : rows[1], csh + t0 : csh + t0 + width],
                    )
    nc.finalize()
    return nc


def _build_weights(F, H, Q, R):
    """Host-side precompute (init-time work in the torch module).

    lhsT[k, m]: out[m, c] = sum_k lhsT[k, m] * rhs[k, c].
    k rows: 0-39 past (8g+j), 40-79 cur, 80-99 y (4g+m).
    m cols: 0-39 m1 (8g+i), 40-79 m2 (skewed), 80-119 m3.
    """
    F64 = np.asarray(F, np.float64)
    H64 = np.asarray(H, np.float64)
    Qinv = np.linalg.inv(np.asarray(Q, np.float64))
    Rinv = np.linalg.inv(np.asarray(R, np.float64))
    A1 = -Qinv
    B1 = Qinv @ F64
    A2 = -(F64.T @ Qinv @ F64)
    B2 = F64.T @ Qinv
    C3 = H64.T @ Rinv          # (D, M)
    A3 = -(C3 @ H64)

    eye = np.eye(NG)
    w = np.zeros((100, 128), NPBF16)
    w[0:40, 0:40] = np.kron(eye, B1.T).astype(NPBF16)      # past -> m1
    w[0:40, 40:80] = np.kron(eye, A2.T).astype(NPBF16)     # past -> m2[T-1]
    w[40:80, 0:40] = np.kron(eye, A1.T).astype(NPBF16)     # cur  -> m1
    w[40:80, 40:80] = np.kron(eye, B2.T).astype(NPBF16)    # cur  -> m2[T-1]
    w[40:80, 80:120] = np.kron(eye, A3.T).astype(NPBF16)   # cur  -> m3
    w[80:100, 80:120] = np.kron(eye, C3.T).astype(NPBF16)  # y    -> m3
    return w


def _pack_inputs(xs, ys, s):
    """xs (nb, D, s), ys (nb, s, M) f32 -> device images (bf16).

    xp[b, 8g+j, c] = xs[b, j, clip(g*gw + c - 1)]   (c in [0, xc))
    yp[b, 4g+m, c] = ys[b, clip(g*gw + c - 1), m]
    """
    gw, xc = _geom(s)
    nb = xs.shape[0]
    cols = np.clip(
        np.arange(NG)[:, None] * gw + np.arange(xc)[None, :] - 1, 0, s - 1
    )  # (NG, xc)
    xs_bf = np.asarray(xs, np.float32).astype(NPBF16)
    xp = xs_bf[:, :, cols]                        # (nb, D, NG, xc)
    xp = np.ascontiguousarray(np.swapaxes(xp, 1, 2)).reshape(nb, 40, xc)

    ys_bf = np.swapaxes(np.asarray(ys, np.float32).astype(NPBF16), 1, 2)  # (nb, M, s)
    yp = ys_bf[:, :, cols]                        # (nb, M, NG, xc)
    yp = np.ascontiguousarray(np.swapaxes(yp, 1, 2)).reshape(nb, 20, xc)
    return xp, yp


_CACHE = {}


def _get_nc(bc=BC, s=S):
    key = (bc, s, os.environ.get("KERNEL_VARIANT"), os.environ.get("KERNEL_SHIFT"))
    if key not in _CACHE:
        _CACHE[key] = _build_nc(bc, s)
    return _CACHE[key]


def run(xs, ys, F, H, Q, R, trace=False, bc=BC, s=S):
    """Shard across 8 cores, run, gather.  Returns ((m1, m2, m3), results)."""
    nb = xs.shape[0]
    assert nb == bc * N_CORES and xs.shape[1:] == (D, s), xs.shape
    assert ys.shape == (nb, s, M), ys.shape
    xp, yp = _pack_inputs(xs, ys, s)
    w_all = _build_weights(F, H, Q, R)

    nc = _get_nc(bc, s)
    in_maps = [
        {
            "xp": np.ascontiguousarray(xp[i * bc : (i + 1) * bc]),
            "yp": np.ascontiguousarray(yp[i * bc : (i + 1) * bc]),
            "w_all": w_all,
        }
        for i in range(N_CORES)
    ]
    res = run_bass_kernel_spmd(nc, in_maps, core_ids=list(range(N_CORES)), trace=trace)
    m_full = np.concatenate([r["m_all"] for r in res.results], axis=0)  # (B,D,3,s)
    outs = tuple(
        np.ascontiguousarray(m_full[:, :, i, :]).astype(np.float32) for i in range(3)
    )
    return outs, res


def kernel(xs, ys, F, H, Q, R):
    trace = bool(int(os.environ.get("KERNEL_TRACE", "0")))
    outs, _ = run(xs, ys, F, H, Q, R, trace=trace)
    return outs


# revision 8
# speedup vs baseline: 1.1672x; 1.1672x over previous
"""Trainium2 Bass kernel for the Kalman graphical-model message-passing problem.

reference math (B=64, D=8, M=4, S=50000):
    m1 = -Qinv @ (xs - F @ x_past)            (B, D, S)
    m2 = FtQinv @ (x_fut - F @ xs)            (B, D, S)
    m3 = HtRinv @ ys_t - (HtRinv @ H) @ xs    (B, D, S)
with x_past/x_fut edge-replicated 1-sample shifts of xs along S.

Design v3 ("NG8 skewed pass", bf16 on the wire; rel-err gate is 2e-2,
bf16 end-to-end measures ~7.5e-3):

  * m1+m2 come from ONE full 128x128 matmul per column chunk.  Per group
    the rhs partition stack is [x[T-1] (8); x[T] (8)] (8 groups = 128
    partitions) and the output stack is [m1[T] (64) | m2[T-1] (64)]:
        m1[T]   = B1 x[T-1] + A1 x[T]
        m2[T-1] = A2 x[T-1] + B2 x[T]    <- skew: m2 pops out one column
    early; its drain writes one column left so stores stay rectangular.
    m3 = A3 x[T] + C3 y[T] is a second 96->64 matmul off the load tile.
    PE: 2*(gw+1) = 12502 cols/batch vs 18750 for the v1 6-matmul variant,
    and 2 LDWEIGHTS per 1024 cols vs 6-7 per 512.

  * DMA width rule (measured: per-DMA throughput scales with partitions
    touched; 40-partition DMAs collapsed to ~90-120 GB/s): every HBM
    transfer here spans 64-96 partitions.  One 96-partition load per
    batch ([x-cur 0-63 | y 64-95], host-packed, 1.2MB), three
    64-partition stores.  The [past|cur] rhs tile is built on-device by
    two DVE copies (bf16 SBUF->SBUF runs in 4x mode, ~0.8us each) with
    32-aligned partition bases -- no extra HBM bytes, no narrow DMAs.

  * PSUM: [128,1024] (mm12) + [64,1024] (mm3) fp32 tiles, 2 bufs each =
    all 8 banks.  Drains (PSUM src = 1x mode) alternate DVE/ACT.

  * Rings: loads on ACT HWDGE, m1 on SP, m2 on gpsimd SWDGE, m3
    alternates per batch -> ~9.6MB per ring.
"""

import os
from contextlib import ExitStack

import ml_dtypes
import numpy as np

import concourse.bacc as bacc
import concourse.bass as bass
import concourse.mybir as mybir
import concourse.tile as tile
from concourse.bass_utils import run_bass_kernel_spmd

F32 = mybir.dt.float32
BF16 = mybir.dt.bfloat16
NPBF16 = ml_dtypes.bfloat16

B, D, M, S = 64, 8, 4, 50000
N_CORES = 8
BC = B // N_CORES  # batches per core
NG = 8             # groups: 8 x 16 m1m2-output states = 128 PSUM partitions
MW = 512           # matmul free-dim (one PSUM bank of fp32)
PW = 2 * MW        # psum tile width (2 banks)


def _geom(s):
    assert s % NG == 0, s
    gw = s // NG       # samples per group
    xc = gw + 2        # image cols c in [0, gw+2); col c ~ sample g*gw+c-1
    return gw, xc


def _build_nc(bc=BC, s=S):
    variant = os.environ.get("KERNEL_VARIANT", "full")  # perf bisection only
    gw, xc = _geom(s)

    nc = bacc.Bacc(trn_type="TRN2")
    vp = nc.dram_tensor("vp", [bc, 96, xc], BF16, kind="ExternalInput")
    w = nc.dram_tensor("w_all", [128, 192], BF16, kind="ExternalInput")
    m_all = nc.dram_tensor("m_all", [bc, D, 3, s], BF16, kind="ExternalOutput")

    with tile.TileContext(nc) as tc, ExitStack() as ctx:
        singles = ctx.enter_context(tc.tile_pool(name="singles", bufs=1))
        lpool = ctx.enter_context(tc.tile_pool(name="l", bufs=3))
        vpool = ctx.enter_context(tc.tile_pool(name="v", bufs=3))
        o12pool = ctx.enter_context(tc.tile_pool(name="o12", bufs=3))
        o3pool = ctx.enter_context(tc.tile_pool(name="o3", bufs=3))
        pp12 = ctx.enter_context(tc.tile_pool(name="pp12", bufs=2, space="PSUM"))
        pp3 = ctx.enter_context(tc.tile_pool(name="pp3", bufs=2, space="PSUM"))

        w_sb = singles.tile([128, 192], BF16, tag="w")
        nc.sync.dma_start(out=w_sb[:], in_=w[:, :])
        w12 = w_sb[0:128, 0:128]
        w3 = w_sb[0:96, 128:192]

        for b in range(bc):
            # load tile: rows 0-63 x-cur (8g+j), rows 64-95 y (4g+m); it is
            # also the mm3 rhs ([cur | y] contraction).
            l_t = lpool.tile([96, xc], BF16, tag="l", name=f"l_{b}")
            nc.scalar.dma_start(
                out=l_t[:], in_=bass.AP(vp, b * 96 * xc, [[xc, 96], [1, xc]])
            )
            if variant == "loads":
                continue
            # mm12 rhs: rows 0-63 past, 64-127 cur (DVE 4x-mode copies;
            # partition bases 0/64 are 32-aligned as the verifier demands).
            v_t = vpool.tile([128, xc], BF16, tag="v", name=f"v_{b}")
            nc.vector.tensor_copy(out=v_t[64:128, 0:xc], in_=l_t[0:64, 0:xc])
            nc.vector.tensor_copy(out=v_t[0:64, 1:xc], in_=l_t[0:64, 0 : xc - 1])

            o12_t = o12pool.tile([128, xc], BF16, tag="o12", name=f"o12_{b}")
            o3_t = o3pool.tile([64, xc], BF16, tag="o3", name=f"o3_{b}")

            n_cols = gw + 1  # compute cols c in [1, gw+2)
            pt = 0
            for c0 in range(1, 1 + n_cols, PW):
                tw = min(PW, 1 + n_cols - c0)
                p12 = pp12.tile([128, PW], F32, tag="p12", name=f"p12_{b}_{c0}")
                p3 = pp3.tile([64, PW], F32, tag="p3", name=f"p3_{b}_{c0}")
                for h0 in range(0, tw, MW):
                    hw = min(MW, tw - h0)
                    nc.tensor.matmul(
                        p12[:, h0 : h0 + hw],
                        w12,
                        v_t[:, c0 + h0 : c0 + h0 + hw],
                        start=True,
                        stop=True,
                    )
                for h0 in range(0, tw, MW):
                    hw = min(MW, tw - h0)
                    nc.tensor.matmul(
                        p3[:, h0 : h0 + hw],
                        w3,
                        l_t[:, c0 + h0 : c0 + h0 + hw],
                        start=True,
                        stop=True,
                    )
                # drains: m1 rows at col c, m2 rows shifted one left so the
                # o12 tile is sample-aligned for a single 128-part store.
                e12, e3 = (nc.vector, nc.scalar) if pt % 2 else (nc.scalar, nc.vector)
                if e12 is nc.vector:
                    nc.vector.tensor_copy(out=o12_t[0:64, c0 : c0 + tw], in_=p12[0:64, 0:tw])
                    nc.vector.tensor_copy(
                        out=o12_t[64:128, c0 - 1 : c0 - 1 + tw], in_=p12[64:128, 0:tw]
                    )
                    nc.scalar.copy(out=o3_t[:, c0 : c0 + tw], in_=p3[:, 0:tw])
                else:
                    nc.scalar.copy(out=o12_t[0:64, c0 : c0 + tw], in_=p12[0:64, 0:tw])
                    nc.scalar.copy(
                        out=o12_t[64:128, c0 - 1 : c0 - 1 + tw], in_=p12[64:128, 0:tw]
                    )
                    nc.vector.tensor_copy(out=o3_t[:, c0 : c0 + tw], in_=p3[:, 0:tw])
                pt += 1

            if variant == "nostores":
                continue
            ooff = b * D * 3 * s
            # m1+m2 go out as ONE 128-partition DMA per column segment (the
            # band dim strides s between m1 and m2); m3 as a 64-partition
            # DMA.  Two segments so segment A streams while the tail
            # computes.  m3's ring alternates per batch to balance bytes.
            seg = min(max(PW, (gw // (2 * PW)) * PW), gw)  # segment-A width
            e12, e3 = (nc.sync, nc.gpsimd) if b % 2 == 0 else (nc.gpsimd, nc.sync)
            for t0, width in ((0, seg), (seg, gw - seg)):
                if width == 0:
                    continue
                e12.dma_start(
                    out=bass.AP(
                        m_all,
                        ooff + t0,
                        [[s, 2], [gw, NG], [3 * s, D], [1, width]],
                    ),
                    in_=o12_t[:, 1 + t0 : 1 + t0 + width],
                )
                e3.dma_start(
                    out=bass.AP(
                        m_all,
                        ooff + 2 * s + t0,
                        [[gw, NG], [3 * s, D], [1, width]],
                    ),
                    in_=o3_t[:, 1 + t0 : 1 + t0 + width],
                )
    nc.finalize()
    return nc


def _build_weights(F, H, Q, R):
    """Host-side precompute (init-time work in the torch module).

    lhsT[k, m]: out[m, c] = sum_k lhsT[k, m] * rhs[k, c].
    w12 (k: 0-63 past 8g+j, 64-127 cur; m: 0-63 m1, 64-127 m2-skewed),
    w3  (k: 0-63 cur, 64-95 y 4g+m; m: 0-63 m3).
    """
    F64 = np.asarray(F, np.float64)
    H64 = np.asarray(H, np.float64)
    Qinv = np.linalg.inv(np.asarray(Q, np.float64))
    Rinv = np.linalg.inv(np.asarray(R, np.float64))
    A1 = -Qinv
    B1 = Qinv @ F64
    A2 = -(F64.T @ Qinv @ F64)
    B2 = F64.T @ Qinv
    C3 = H64.T @ Rinv          # (D, M)
    A3 = -(C3 @ H64)

    eye = np.eye(NG)
    w = np.zeros((128, 192), NPBF16)
    w[0:64, 0:64] = np.kron(eye, B1.T).astype(NPBF16)       # past -> m1
    w[0:64, 64:128] = np.kron(eye, A2.T).astype(NPBF16)     # past -> m2[T-1]
    w[64:128, 0:64] = np.kron(eye, A1.T).astype(NPBF16)     # cur  -> m1
    w[64:128, 64:128] = np.kron(eye, B2.T).astype(NPBF16)   # cur  -> m2[T-1]
    w[0:64, 128:192] = np.kron(eye, A3.T).astype(NPBF16)    # cur  -> m3
    w[64:96, 128:192] = np.kron(eye, C3.T).astype(NPBF16)   # y    -> m3
    return w


def _pack_inputs(xs, ys, s):
    """xs (nb, D, s), ys (nb, s, M) f32 -> one device image (bf16).

    vp[b, 8g+j, c]      = xs[b, j, clip(g*gw + c - 1)]   (c in [0, xc))
    vp[b, 64+4g+m, c]   = ys[b, clip(g*gw + c - 1), m]
    """
    gw, xc = _geom(s)
    nb = xs.shape[0]
    cols = np.clip(
        np.arange(NG)[:, None] * gw + np.arange(xc)[None, :] - 1, 0, s - 1
    )  # (NG, xc)
    vp = np.empty((nb, 96, xc), NPBF16)
    xs_bf = np.asarray(xs, np.float32).astype(NPBF16)
    xp = xs_bf[:, :, cols]                        # (nb, D, NG, xc)
    vp[:, 0:64] = np.swapaxes(xp, 1, 2).reshape(nb, 64, xc)
    ys_bf = np.swapaxes(np.asarray(ys, np.float32).astype(NPBF16), 1, 2)  # (nb, M, s)
    yp = ys_bf[:, :, cols]                        # (nb, M, NG, xc)
    vp[:, 64:96] = np.swapaxes(yp, 1, 2).reshape(nb, 32, xc)
    return vp


_CACHE = {}


def _get_nc(bc=BC, s=S):
    key = (bc, s, os.environ.get("KERNEL_VARIANT"))
    if key not in _CACHE:
        _CACHE[key] = _build_nc(bc, s)
    return _CACHE[key]


def run(xs, ys, F, H, Q, R, trace=False, bc=BC, s=S):
    """Shard across 8 cores, run, gather.  Returns ((m1, m2, m3), results)."""
    nb = xs.shape[0]
    assert nb == bc * N_CORES and xs.shape[1:] == (D, s), xs.shape
    assert ys.shape == (nb, s, M), ys.shape
    vp = _pack_inputs(xs, ys, s)
    w_all = _build_weights(F, H, Q, R)

    nc = _get_nc(bc, s)
    in_maps = [
        {
            "vp": np.ascontiguousarray(vp[i * bc : (i + 1) * bc]),
            "w_all": w_all,
        }
        for i in range(N_CORES)
    ]
    res = run_bass_kernel_spmd(nc, in_maps, core_ids=list(range(N_CORES)), trace=trace)
    m_full = np.concatenate([r["m_all"] for r in res.results], axis=0)  # (B,D,3,s)
    outs = tuple(
        np.ascontiguousarray(m_full[:, :, i, :]).astype(np.float32) for i in range(3)
    )
    return outs, res


def kernel(xs, ys, F, H, Q, R):
    trace = bool(int(os.environ.get("KERNEL_TRACE", "0")))
    outs, _ = run(xs, ys, F, H, Q, R, trace=trace)
    return outs
